# revision 1
# baseline (speedup 1.0000x reference)
"""Channel-attention kernel for Trainium2 (8 NeuronCores, data-parallel over batch).

Math: the reference expands x (B,C,T,1) to 8 channels via a 1x1 conv before the
Q@K^T einsum.  Algebraically, with alpha=w1.w2, beta=w1.b2, delta=b1.w2,
eta=b1.b2 and s[b,c]=sum_t x[b,c,t]:

    energy[b,c,e] = alpha*G[b,c,e] + beta*s[b,c] + delta*s[b,e] + T*eta
    G[b] = X[b] @ X[b]^T          (X[b] = x[b,:,:,0], shape (C,T))

The beta*s[c] and T*eta terms are constant along the e (last) axis, so they
cancel in the min-max normalization; only alpha*G + delta*s_e matters.  This
cuts the contraction from T*8 down to T (the advertised 8x headroom).

Per core: 8 batches, processed as 4 pairs of 2 batches stacked on the 128
partitions.  Per pair:
  - load X2 (128,4000) f32, cast to bf16 (zero-padded to 4096 cols)
  - xbar DMA-transpose -> Xt: 32 contiguous tiles of (t=128, c=128)
  - Gram matmuls (bf16) accumulate G2; row-sums s come from the cast ops'
    accum_out (computed concurrently on DVE and ACT)
  - s-row via PE transpose; aux matmul adds (delta/alpha)*s_row to psum
  - extract diagonal (64,64) blocks scaled by alpha, min-max norm + softmax
  - attention (block-diag, scaled by gamma) @ X2bf -> psum, + x, store
"""

import numpy as np
import ml_dtypes
from contextlib import ExitStack

import concourse.bass as bass
import concourse.tile as tile
from concourse import mybir
from concourse.bass_utils import run_bass_kernel_spmd
from concourse.alu_op_type import AluOpType

F32 = mybir.dt.float32
BF16 = mybir.dt.bfloat16
AX = mybir.AxisListType.X

B, C, T = 64, 64, 4000
NCORES = 8
BPC = B // NCORES          # 8 batches per core
PAIRS = BPC // 2           # 4 pairs of 2 batches
ROWS = BPC * C             # 512 rows of (C,T) per core
TP = 4096                  # T padded to a multiple of 128
NKT = TP // 128            # 32 k-tiles
NCHUNK = 8
CHW = T // NCHUNK          # 500 (fits one PSUM bank in f32)
EPS = 1e-8
TSTRIDE = 129              # per-k-tile stride in Xt (128 data cols + 1 ones col)
NTSPLIT = 2                # dma-transposes per pair (pipelining granularity)


def _body(ctx, tc, out_ap, x_ap, idf_ap, alpha, doa, gamma):
    nc = tc.nc

    singles = ctx.enter_context(tc.tile_pool(name="singles", bufs=1))
    xfp = ctx.enter_context(tc.tile_pool(name="xfp", bufs=3))
    xbp = ctx.enter_context(tc.tile_pool(name="xbp", bufs=3))
    xtp = ctx.enter_context(tc.tile_pool(name="xtp", bufs=3))
    obp = ctx.enter_context(tc.tile_pool(name="obp", bufs=2))
    attp = ctx.enter_context(tc.tile_pool(name="attp", bufs=2))
    stage = ctx.enter_context(tc.tile_pool(name="stage", bufs=2))
    smalls = ctx.enter_context(tc.tile_pool(name="smalls", bufs=3))

    ps_s = ctx.enter_context(tc.tile_pool(name="ps_s", bufs=2, space="PSUM"))
    ps_g = ctx.enter_context(tc.tile_pool(name="ps_g", bufs=2, space="PSUM"))
    ps_o = ctx.enter_context(tc.tile_pool(name="ps_o", bufs=4, space="PSUM"))

    ident_f32 = singles.tile([128, 128], F32)
    nc.sync.dma_start(ident_f32[:], idf_ap)
    ones_row = singles.tile([1, 128], BF16)
    nc.vector.memset(ones_row[:], 1.0)
    # preload the ACT function tables during the ramp
    warm_act = singles.tile([1, 2], F32)
    nc.scalar.activation(
        warm_act[:], ones_row[0:1, 0:2], mybir.ActivationFunctionType.Exp
    )

    st = [{} for _ in range(PAIRS)]

    def stage0(p):
        """loads (kept at the head of the HWDGE ring, no head-of-line)."""
        v = st[p]
        rows = slice(p * 128, (p + 1) * 128)
        x_f32 = xfp.tile([128, T], F32)
        half = T // 2
        nc.sync.dma_start(x_f32[:, 0:half], x_ap[rows, 0:half])
        nc.sync.dma_start(x_f32[:, half:T], x_ap[rows, half:T])
        v["x_f32"] = x_f32

    def stage1(p):
        """cast (ACT, with row-sum accumulation) + DMA transposes + the
        s-row prep (so the PE stream later never stalls on it)."""
        v = st[p]
        x_f32 = v["x_f32"]
        x_bf = xbp.tile([128, TP], BF16)
        half = TP // 2  # aligned with the transpose halves
        xt = xtp.tile([128, NKT * 128], BF16)
        ksp = NKT // NTSPLIT

        def transp(h):
            nc.sync.dma_start_transpose(
                xt[:, h * ksp * 128:(h + 1) * ksp * 128].rearrange(
                    "q (k f) -> q k f", f=128
                ),
                x_bf[:, h * ksp * 128:(h + 1) * ksp * 128],
            )

        # the two cast halves run concurrently on DVE and ACT, each also
        # accumulating its half of the row sums s
        s_ab = smalls.tile([128, 2], F32)
        nc.vector.tensor_scalar(
            x_bf[:, 0:half], x_f32[:, 0:half], scalar1=1.0, scalar2=0.0,
            op0=AluOpType.mult, op1=AluOpType.add, accum_out=s_ab[:, 0:1],
        )
        transp(0)
        nc.scalar.activation(
            x_bf[:, half:T], x_f32[:, half:T],
            mybir.ActivationFunctionType.Copy, accum_out=s_ab[:, 1:2],
        )
        nc.vector.memset(x_bf[:, T:TP], 0.0)
        transp(1)
        s_col = smalls.tile([128, 1], F32)
        nc.vector.tensor_reduce(s_col[:], s_ab[:], axis=AX, op=AluOpType.add)
        st_ps = ps_s.tile([1, 128], F32, tag="st")
        nc.tensor.transpose(st_ps[:], s_col[:], ident_f32[:])
        rhs_aux = smalls.tile([1, 128], BF16)
        nc.vector.tensor_scalar_mul(rhs_aux[:], st_ps[:], doa)
        v.update(x_bf=x_bf, xt=xt, rhs_aux=rhs_aux)

    def stage2x(p):
        """Gram matmuls + the aux rank-1 update (pure PE, no stalls)."""
        v = st[p]
        xt = v["xt"]
        psum_g = ps_g.tile([128, 128], F32, tag="g")
        for kt in range(NKT):
            base = kt * 128
            nc.tensor.matmul(
                psum_g[:],
                lhsT=xt[:, base: base + 128],
                rhs=xt[:, base: base + 128],
                start=(kt == 0),
                stop=(kt == NKT - 1),
            )
        nc.tensor.matmul(
            psum_g[:],
            lhsT=ones_row[:],
            rhs=v["rhs_aux"][:],
            start=False,
            stop=True,
            skip_group_check=True,
        )
        v["psum_g"] = psum_g

    def stage2y(p):
        """energy extraction + min-max softmax -> attention lhsT."""
        v = st[p]
        psum_g = v["psum_g"]
        # Diagonal (64,64) blocks, scaled by alpha -> energy (128, 64)
        e_sb = smalls.tile([128, 64], F32)
        nc.vector.tensor_scalar_mul(e_sb[0:64, :], psum_g[0:64, 0:64], alpha)
        nc.vector.tensor_scalar_mul(
            e_sb[64:128, :], psum_g[64:128, 64:128], alpha
        )

        # min-max normalize along free axis, then softmax (normalized values
        # live in [0,1], so no max-subtraction is needed before exp)
        rmax = smalls.tile([128, 1], F32)
        nc.vector.tensor_reduce(rmax[:], e_sb[:], axis=AX, op=AluOpType.max)
        rmin = smalls.tile([128, 1], F32)
        nc.vector.tensor_reduce(rmin[:], e_sb[:], axis=AX, op=AluOpType.min)
        den = smalls.tile([128, 1], F32)
        nc.vector.tensor_scalar(
            den[:], rmax[:], scalar1=rmin[:], scalar2=EPS,
            op0=AluOpType.subtract, op1=AluOpType.add,
        )
        rden = smalls.tile([128, 1], F32)
        nc.vector.reciprocal(rden[:], den[:])
        nbias = smalls.tile([128, 1], F32)
        nc.vector.scalar_tensor_tensor(
            nbias[:], in0=rmin[:], scalar=-1.0, in1=rden[:],
            op0=AluOpType.mult, op1=AluOpType.mult,
        )
        ex = smalls.tile([128, 64], F32)
        nc.scalar.activation(
            ex[:], e_sb[:], mybir.ActivationFunctionType.Exp,
            bias=nbias[:], scale=rden[:],
        )
        ssum = smalls.tile([128, 1], F32)
        nc.vector.tensor_reduce(ssum[:], ex[:], axis=AX, op=AluOpType.add)
        rsum = smalls.tile([128, 1], F32)
        nc.vector.reciprocal(rsum[:], ssum[:])

        latt = attp.tile([128, 128], BF16)
        nc.vector.memset(latt[:], 0.0)
        nc.vector.tensor_scalar(
            latt[0:64, 0:64], ex[0:64, :], scalar1=rsum[0:64], scalar2=gamma,
            op0=AluOpType.mult, op1=AluOpType.mult,
        )
        nc.vector.tensor_scalar(
            latt[64:128, 64:128], ex[64:128, :], scalar1=rsum[64:128],
            scalar2=gamma, op0=AluOpType.mult, op1=AluOpType.mult,
        )
        v["latt"] = latt

    def stage3(p):
        """attended chunks + residual add + store.  Even chunks: DVE adds
        from PSUM (store each immediately).  Odd chunks: ACT copies
        PSUM->SBUF, one GPSIMD op adds them all, one strided store."""
        v = st[p]
        rows = slice(p * 128, (p + 1) * 128)
        x_f32, x_bf, latt = v["x_f32"], v["x_bf"], v["latt"]
        out_sb = obp.tile([128, T], F32)
        att_st = stage.tile([128, 4, CHW], F32)
        last = p == PAIRS - 1
        for ch in range(NCHUNK):
            cols = slice(ch * CHW, (ch + 1) * CHW)
            psum_o = ps_o.tile([128, CHW], F32, tag="o")
            nc.tensor.matmul(
                psum_o[:], lhsT=latt[:], rhs=x_bf[:, cols], start=True,
                stop=True,
            )
            if ch % 2 == 0 or last:
                nc.vector.tensor_add(out_sb[:, cols], psum_o[:], x_f32[:, cols])
            else:
                nc.scalar.copy(att_st[:, ch // 2, :], psum_o[:])
        ev3 = out_sb.rearrange("q (c w) -> q c w", w=CHW)[:, 0::2, :]
        od3 = out_ap[rows, :].rearrange("q (c w) -> q c w", w=CHW)
        if last:
            nc.sync.dma_start(out_ap[rows, :], out_sb[:])
        else:
            nc.sync.dma_start(od3[:, 0::2, :], ev3)
            oddv = out_sb.rearrange("q (c w) -> q c w", w=CHW)[:, 1::2, :]
            xodd = x_f32.rearrange("q (c w) -> q c w", w=CHW)[:, 1::2, :]
            nc.gpsimd.tensor_add(oddv, att_st[:], xodd)
            nc.sync.dma_start(od3[:, 1::2, :], oddv)
        v.clear()

    # software-pipelined schedule, hand-skewed so the PE instruction stream
    # (st-transpose / gram+aux / att) never waits on same-pair DVE/ACT work
    sched = [
        (stage0, 0), (stage1, 0), (stage0, 1), (stage1, 1),
        (stage2x, 0), (stage0, 2), (stage1, 2),
        (stage2x, 1), (stage2y, 0), (stage0, 3), (stage1, 3),
        (stage2x, 2), (stage3, 0), (stage2y, 1),
        (stage2x, 3), (stage3, 1), (stage2y, 2),
        (stage3, 2), (stage2y, 3),
        (stage3, 3),
    ]
    for fn, p in sched:
        fn(p)


_MODULE_CACHE = {}


def _build_module(alpha, doa, gamma):
    key = (alpha, doa, gamma)
    if key in _MODULE_CACHE:
        return _MODULE_CACHE[key]
    nc = bass.Bass(
        "TRN2", target_bir_lowering=False, debug=False, num_devices=NCORES
    )
    x_ap = nc.dram_tensor("x", (ROWS, T), F32, kind="ExternalInput").ap()
    idf_ap = nc.dram_tensor("idf", (128, 128), F32, kind="ExternalInput").ap()
    out_ap = nc.dram_tensor("out", (ROWS, T), F32, kind="ExternalOutput").ap()
    with tile.TileContext(nc) as tc, ExitStack() as ctx:
        _body(ctx, tc, out_ap, x_ap, idf_ap, alpha, doa, gamma)
    if _LEGALIZE_WAITS:
        _split_waits(nc)
    _MODULE_CACHE[key] = nc
    return nc


# The wait-split legalization confuses CoreSim's bookkeeping (hand-built
# NoOps bypass nc.inst_map); tests flip this off for simulation runs.
_LEGALIZE_WAITS = True


def _split_waits(nc):
    """walrus TRN2 codegen allows only ONE sync wait per instruction; when
    Tile emits more (e.g. PSUM slot reuse: previous-writer completion +
    previous-reader), hoist the extras onto same-engine NoOps inserted
    immediately before — the sequencer dispatches in order, so the blocking
    semantics are identical."""
    nid = [0]
    for f in nc.m.functions:
        for block in f.blocks:
            out = []
            for inst in block.instructions:
                si = getattr(inst, "sync_info", None)
                if (
                    si is not None
                    and si.on_wait
                    and len(si.on_wait) > 1
                    and type(inst).__name__ != "InstNoOp"
                ):
                    waits = list(si.on_wait)
                    for w in waits[:-1]:
                        nid[0] += 1
                        out.append(
                            mybir.InstNoOp(
                                name=f"{inst.name}-wsplit{nid[0]}",
                                engine=inst.engine,
                                ins=[],
                                outs=[],
                                sync_info=mybir.SyncInfo(
                                    on_wait=[w], on_update=[]
                                ),
                                text_hint="wait-split",
                                bass_nofuse=True,
                            )
                        )
                    inst.sync_info = mybir.SyncInfo(
                        on_wait=waits[-1:], on_update=list(si.on_update)
                    )
                out.append(inst)
            block.instructions[:] = out


def _prepare(inputs):
    x = np.ascontiguousarray(
        np.asarray(inputs["x"], dtype=np.float32).reshape(B * C, T)
    )
    w1 = np.asarray(inputs["w1"], dtype=np.float64)
    b1 = np.asarray(inputs["b1"], dtype=np.float64)
    w2 = np.asarray(inputs["w2"], dtype=np.float64)
    b2 = np.asarray(inputs["b2"], dtype=np.float64)
    gamma = float(np.asarray(inputs["gamma"]))
    alpha = float(w1 @ w2)
    delta = float(b1 @ w2)
    assert abs(alpha) > 1e-12, "degenerate alpha not supported"
    nc = _build_module(alpha, delta / alpha, gamma)
    ident_f = np.eye(128, dtype=np.float32)
    in_maps = [
        {"x": x[i * ROWS:(i + 1) * ROWS], "idf": ident_f}
        for i in range(NCORES)
    ]
    return nc, in_maps


def kernel(**inputs):
    nc, in_maps = _prepare(inputs)
    res = run_bass_kernel_spmd(nc, in_maps, core_ids=list(range(NCORES)))
    out = np.concatenate([res.results[i]["out"] for i in range(NCORES)], axis=0)
    return out.reshape(B, C, T, 1)



# revision 5
# speedup vs baseline: 1.6182x; 1.6182x over previous
"""Channel-attention kernel for Trainium2 (8 NeuronCores, data-parallel over batch).

Math: the reference expands x (B,C,T,1) to 8 channels via a 1x1 conv before the
Q@K^T einsum.  Algebraically, with alpha=w1.w2, delta=b1.w2 and
s[b,c]=sum_t x[b,c,t]:

    energy[b,c,e] = alpha*G[b,c,e] + delta*s[b,e] + (terms constant in e)
    G[b] = X[b] @ X[b]^T          (X[b] = x[b,:,:,0], shape (C,T))

Row-constant terms cancel in the min-max normalization, so only
alpha*G + delta*s_e matters.

Per core: 8 batches, processed as 4 pairs of 2 batches stacked on the 128
partitions.  v2 design (vs the DMA-transpose baseline):
  - transposes happen on the PE (32 (128,128) bf16 transposes per pair into
    PSUM, batch-copied to SBUF) -- no xbar DMA traffic at all
  - the residual add is folded into the attention matmul by adding the
    identity to the (gamma-scaled) attention weights: out = (gamma*A + I)^T X
    so x_f32 is dead right after the bf16 cast
  - output is stored as bf16 (half the write traffic), converted on host
  - x loads go on the SP HWDGE ring, output stores on the ACT ring
HBM traffic per core: 8.2 MB read + 4.1 MB write (vs 20.6 MB total DMA before).
"""

import numpy as np
import ml_dtypes
from contextlib import ExitStack

import concourse.bass as bass
import concourse.tile as tile
from concourse import mybir
from concourse.bass_utils import run_bass_kernel_spmd
from concourse.alu_op_type import AluOpType

F32 = mybir.dt.float32
BF16 = mybir.dt.bfloat16
AX = mybir.AxisListType.X

B, C, T = 64, 64, 4000
NCORES = 8
BPC = B // NCORES          # 8 batches per core
PAIRS = BPC // 2           # 4 pairs of 2 batches
ROWS = BPC * C             # 512 rows of (C,T) per core
TP = 4096                  # T padded to a multiple of 128
NKT = TP // 128            # 32 k-tiles
GT = 8                     # transpose tiles per PSUM group
NG = NKT // GT             # 4 groups per pair
GW = GT * 128              # 1024 columns per group
NCHUNK = 8
CHW = T // NCHUNK          # 500 (fits one PSUM bank in f32)
CSPLIT = 2048              # load/cast split point
EPS = 1e-8


def _body(ctx, tc, out_ap, x_ap, idf_ap, idb_ap, alpha, doa, gamma):
    nc = tc.nc

    singles = ctx.enter_context(tc.tile_pool(name="singles", bufs=1))
    xfp = ctx.enter_context(tc.tile_pool(name="xfp", bufs=2))
    xbp = ctx.enter_context(tc.tile_pool(name="xbp", bufs=3))
    xtp = ctx.enter_context(tc.tile_pool(name="xtp", bufs=2))
    obp = ctx.enter_context(tc.tile_pool(name="obp", bufs=2))
    attp = ctx.enter_context(tc.tile_pool(name="attp", bufs=2))
    smalls = ctx.enter_context(tc.tile_pool(name="smalls", bufs=3))

    ps_t = ctx.enter_context(tc.tile_pool(name="ps_t", bufs=2, space="PSUM"))
    ps_g = ctx.enter_context(tc.tile_pool(name="ps_g", bufs=2, space="PSUM"))
    ps_o = ctx.enter_context(tc.tile_pool(name="ps_o", bufs=3, space="PSUM"))
    ps_s = ctx.enter_context(tc.tile_pool(name="ps_s", bufs=1, space="PSUM"))

    st = [{} for _ in range(PAIRS)]

    def ld(p):
        v = st[p]
        rows = slice(p * 128, (p + 1) * 128)
        x_f32 = xfp.tile([128, T], F32)
        nc.sync.dma_start(x_f32[:, 0:CSPLIT], x_ap[rows, 0:CSPLIT])
        nc.sync.dma_start(x_f32[:, CSPLIT:T], x_ap[rows, CSPLIT:T])
        v["x_f32"] = x_f32

    # identities and constants ride the ACT ring so the SP ring is pure loads
    ident_f32 = singles.tile([128, 128], F32)
    ident_bf = singles.tile([128, 128], BF16)
    ones_row = singles.tile([1, 128], BF16)

    def preamble():
        nc.scalar.dma_start(ident_bf[:], idb_ap)
        nc.scalar.dma_start(ident_f32[:], idf_ap)
        nc.vector.memset(ones_row[:], 1.0)
        warm_act = singles.tile([1, 2], F32)
        nc.scalar.activation(
            warm_act[:], ones_row[0:1, 0:2], mybir.ActivationFunctionType.Exp
        )

    def cast(p):
        """f32 -> bf16 on DVE/ACT halves, accumulating row sums s."""
        v = st[p]
        x_f32 = v["x_f32"]
        x_bf = xbp.tile([128, TP], BF16)
        s_ab = smalls.tile([128, 2], F32)
        nc.gpsimd.memset(x_bf[:, T:TP], 0.0)
        nc.vector.tensor_scalar(
            x_bf[:, 0:CSPLIT], x_f32[:, 0:CSPLIT], scalar1=1.0, scalar2=0.0,
            op0=AluOpType.mult, op1=AluOpType.add, accum_out=s_ab[:, 0:1],
        )
        nc.scalar.activation(
            x_bf[:, CSPLIT:T], x_f32[:, CSPLIT:T],
            mybir.ActivationFunctionType.Copy, accum_out=s_ab[:, 1:2],
        )
        s_col = smalls.tile([128, 1], F32)
        nc.vector.tensor_reduce(s_col[:], s_ab[:], axis=AX, op=AluOpType.add)
        v.update(x_bf=x_bf, s_col=s_col)

    def tgrp(p, g):
        """8 PE transposes into one PSUM bank, then one batched copy out."""
        v = st[p]
        x_bf = v["x_bf"]
        if g == 0:
            xt = xtp.tile([128, TP], BF16)
            v["xt"] = xt
        ps = ps_t.tile([128, GW], BF16, tag="t")
        for j in range(GT):
            kt = g * GT + j
            nc.tensor.transpose(
                ps[:, j * 128:(j + 1) * 128],
                x_bf[:, kt * 128:(kt + 1) * 128],
                ident_bf[:],
            )
        dst = v["xt"][:, g * GW:(g + 1) * GW]
        if g % 2 == 0:
            nc.vector.tensor_copy(dst, ps[:])
        else:
            nc.scalar.copy(dst, ps[:])

    def gram(p):
        """s-row transpose + 32 Gram matmuls + aux rank-1 update (pure PE)."""
        v = st[p]
        st_ps = ps_s.tile([1, 128], F32, tag="st")
        nc.tensor.transpose(st_ps[:], v["s_col"][:], ident_f32[:])
        rhs_aux = smalls.tile([1, 128], BF16)
        nc.vector.tensor_scalar_mul(rhs_aux[:], st_ps[:], doa)
        xt = v["xt"]
        psum_g = ps_g.tile([128, 128], F32, tag="g")
        for kt in range(NKT):
            base = kt * 128
            nc.tensor.matmul(
                psum_g[:],
                lhsT=xt[:, base: base + 128],
                rhs=xt[:, base: base + 128],
                start=(kt == 0),
                stop=(kt == NKT - 1),
            )
        nc.tensor.matmul(
            psum_g[:],
            lhsT=ones_row[:],
            rhs=rhs_aux[:],
            start=False,
            stop=True,
            skip_group_check=True,
        )
        v["psum_g"] = psum_g

    def smax(p):
        """energy extraction + min-max softmax -> attention lhsT (+identity)."""
        v = st[p]
        psum_g = v["psum_g"]
        e_sb = smalls.tile([128, 64], F32)
        nc.vector.tensor_scalar_mul(e_sb[0:64, :], psum_g[0:64, 0:64], alpha)
        nc.vector.tensor_scalar_mul(
            e_sb[64:128, :], psum_g[64:128, 64:128], alpha
        )
        rmax = smalls.tile([128, 1], F32)
        nc.vector.tensor_reduce(rmax[:], e_sb[:], axis=AX, op=AluOpType.max)
        rmin = smalls.tile([128, 1], F32)
        nc.vector.tensor_reduce(rmin[:], e_sb[:], axis=AX, op=AluOpType.min)
        den = smalls.tile([128, 1], F32)
        nc.vector.tensor_scalar(
            den[:], rmax[:], scalar1=rmin[:], scalar2=EPS,
            op0=AluOpType.subtract, op1=AluOpType.add,
        )
        rden = smalls.tile([128, 1], F32)
        nc.vector.reciprocal(rden[:], den[:])
        nbias = smalls.tile([128, 1], F32)
        nc.vector.scalar_tensor_tensor(
            nbias[:], in0=rmin[:], scalar=-1.0, in1=rden[:],
            op0=AluOpType.mult, op1=AluOpType.mult,
        )
        ex = smalls.tile([128, 64], F32)
        nc.scalar.activation(
            ex[:], e_sb[:], mybir.ActivationFunctionType.Exp,
            bias=nbias[:], scale=rden[:],
        )
        ssum = smalls.tile([128, 1], F32)
        nc.vector.tensor_reduce(ssum[:], ex[:], axis=AX, op=AluOpType.add)
        rsum = smalls.tile([128, 1], F32)
        nc.vector.reciprocal(rsum[:], ssum[:])

        latt = attp.tile([128, 128], BF16)
        nc.vector.memset(latt[:], 0.0)
        nc.vector.tensor_scalar(
            latt[0:64, 0:64], ex[0:64, :], scalar1=rsum[0:64], scalar2=gamma,
            op0=AluOpType.mult, op1=AluOpType.mult,
        )
        nc.vector.tensor_scalar(
            latt[64:128, 64:128], ex[64:128, :], scalar1=rsum[64:128],
            scalar2=gamma, op0=AluOpType.mult, op1=AluOpType.mult,
        )
        # residual: out = (gamma*A + I)^T @ X
        nc.vector.tensor_add(latt[:], latt[:], ident_bf[:])
        v["latt"] = latt

    def attn(p):
        """attention+residual matmul chunks -> bf16 out tile -> ACT-ring store."""
        v = st[p]
        rows = slice(p * 128, (p + 1) * 128)
        x_bf, latt = v["x_bf"], v["latt"]
        out_bf = obp.tile([128, T], BF16)
        for ch in range(NCHUNK):
            cols = slice(ch * CHW, (ch + 1) * CHW)
            psum_o = ps_o.tile([128, CHW], F32, tag="o")
            nc.tensor.matmul(
                psum_o[:], lhsT=latt[:], rhs=x_bf[:, cols], start=True,
                stop=True,
            )
            if ch % 2 == 0:
                nc.vector.tensor_copy(out_bf[:, cols], psum_o[:])
            else:
                nc.scalar.copy(out_bf[:, cols], psum_o[:])
            if ch == 3:
                nc.sync.dma_start(
                    out_ap[rows, 0:2000], out_bf[:, 0:2000]
                )
        nc.sync.dma_start(out_ap[rows, 2000:T], out_bf[:, 2000:T])
        v.clear()

    def tall(p):
        for g in range(NG):
            tgrp(p, g)

    # software-pipelined schedule: PE order is
    #   T0 g0 T1 a0 g1 T2 a1 g2 T3 a2 g3 a3  (dense), with the batched
    # PSUM->SBUF copies, softmax and out-copies filling DVE/ACT/Pool slots.
    sched = [
        (ld, 0), (preamble, None), (cast, 0), (ld, 1),
        (tall, 0), (cast, 1), (ld, 2),
        (gram, 0), (tall, 1), (smax, 0), (cast, 2), (ld, 3),
        (attn, 0), (gram, 1), (tall, 2), (smax, 1), (cast, 3),
        (attn, 1), (gram, 2), (tall, 3), (smax, 2),
        (attn, 2), (gram, 3), (smax, 3),
        (attn, 3),
    ]
    for fn, p in sched:
        if p is None:
            fn()
        else:
            fn(p)


_MODULE_CACHE = {}


def _build_module(alpha, doa, gamma):
    key = (alpha, doa, gamma)
    if key in _MODULE_CACHE:
        return _MODULE_CACHE[key]
    nc = bass.Bass(
        "TRN2", target_bir_lowering=False, debug=False, num_devices=NCORES
    )
    x_ap = nc.dram_tensor("x", (ROWS, T), F32, kind="ExternalInput").ap()
    idf_ap = nc.dram_tensor("idf", (128, 128), F32, kind="ExternalInput").ap()
    idb_ap = nc.dram_tensor("idb", (128, 128), BF16, kind="ExternalInput").ap()
    out_ap = nc.dram_tensor("out", (ROWS, T), BF16, kind="ExternalOutput").ap()
    with tile.TileContext(nc) as tc, ExitStack() as ctx:
        _body(ctx, tc, out_ap, x_ap, idf_ap, idb_ap, alpha, doa, gamma)
    if _LEGALIZE_WAITS:
        _split_waits(nc)
    _MODULE_CACHE[key] = nc
    return nc


# The wait-split legalization confuses CoreSim's bookkeeping (hand-built
# NoOps bypass nc.inst_map); tests flip this off for simulation runs.
_LEGALIZE_WAITS = True


def _split_waits(nc):
    """walrus TRN2 codegen allows only ONE sync wait per instruction; when
    Tile emits more (e.g. PSUM slot reuse: previous-writer completion +
    previous-reader), hoist the extras onto same-engine NoOps inserted
    immediately before — the sequencer dispatches in order, so the blocking
    semantics are identical."""
    nid = [0]
    for f in nc.m.functions:
        for block in f.blocks:
            out = []
            for inst in block.instructions:
                si = getattr(inst, "sync_info", None)
                if (
                    si is not None
                    and si.on_wait
                    and len(si.on_wait) > 1
                    and type(inst).__name__ != "InstNoOp"
                ):
                    waits = list(si.on_wait)
                    for w in waits[:-1]:
                        nid[0] += 1
                        out.append(
                            mybir.InstNoOp(
                                name=f"{inst.name}-wsplit{nid[0]}",
                                engine=inst.engine,
                                ins=[],
                                outs=[],
                                sync_info=mybir.SyncInfo(
                                    on_wait=[w], on_update=[]
                                ),
                                text_hint="wait-split",
                                bass_nofuse=True,
                            )
                        )
                    inst.sync_info = mybir.SyncInfo(
                        on_wait=waits[-1:], on_update=list(si.on_update)
                    )
                out.append(inst)
            block.instructions[:] = out


def _prepare(inputs):
    x = np.ascontiguousarray(
        np.asarray(inputs["x"], dtype=np.float32).reshape(B * C, T)
    )
    w1 = np.asarray(inputs["w1"], dtype=np.float64)
    b1 = np.asarray(inputs["b1"], dtype=np.float64)
    w2 = np.asarray(inputs["w2"], dtype=np.float64)
    b2 = np.asarray(inputs["b2"], dtype=np.float64)
    gamma = float(np.asarray(inputs["gamma"]))
    alpha = float(w1 @ w2)
    delta = float(b1 @ w2)
    assert abs(alpha) > 1e-12, "degenerate alpha not supported"
    nc = _build_module(alpha, delta / alpha, gamma)
    ident_f = np.eye(128, dtype=np.float32)
    ident_b = np.eye(128, dtype=ml_dtypes.bfloat16)
    in_maps = [
        {"x": x[i * ROWS:(i + 1) * ROWS], "idf": ident_f, "idb": ident_b}
        for i in range(NCORES)
    ]
    return nc, in_maps


def kernel(**inputs):
    nc, in_maps = _prepare(inputs)
    res = run_bass_kernel_spmd(nc, in_maps, core_ids=list(range(NCORES)))
    out = np.concatenate(
        [np.asarray(res.results[i]["out"]) for i in range(NCORES)], axis=0
    ).astype(np.float32)
    return out.reshape(B, C, T, 1)


# revision 9
# speedup vs baseline: 1.7189x; 1.0623x over previous
"""Channel-attention kernel for Trainium2 (8 NeuronCores, data-parallel over batch).

Math: the reference expands x (B,C,T,1) to 8 channels via a 1x1 conv before the
Q@K^T einsum.  Algebraically, with alpha=w1.w2, delta=b1.w2 and
s[b,c]=sum_t x[b,c,t]:

    energy[b,c,e] = alpha*G[b,c,e] + delta*s[b,e] + (terms constant in e)
    G[b] = X[b] @ X[b]^T          (X[b] = x[b,:,:,0], shape (C,T))

Row-constant terms cancel in the min-max normalization, so only
alpha*G + delta*s_e matters.

Per core: 8 batches, processed as 4 pairs of 2 batches stacked on the 128
partitions.  v2 design (vs the DMA-transpose baseline):
  - transposes happen on the PE (32 (128,128) bf16 transposes per pair into
    PSUM, batch-copied to SBUF) -- no xbar DMA traffic at all
  - the residual add is folded into the attention matmul by adding the
    identity to the (gamma-scaled) attention weights: out = (gamma*A + I)^T X
    so x_f32 is dead right after the bf16 cast
  - output is stored as bf16 (half the write traffic), converted on host
  - x loads go on the SP HWDGE ring, output stores on the ACT ring
HBM traffic per core: 8.2 MB read + 4.1 MB write (vs 20.6 MB total DMA before).
"""

import numpy as np
import ml_dtypes
from contextlib import ExitStack

import concourse.bass as bass
import concourse.tile as tile
from concourse import mybir
from concourse.bass_utils import run_bass_kernel_spmd
from concourse.alu_op_type import AluOpType

F32 = mybir.dt.float32
BF16 = mybir.dt.bfloat16
AX = mybir.AxisListType.X

B, C, T = 64, 64, 4000
NCORES = 8
BPC = B // NCORES          # 8 batches per core
PAIRS = BPC // 2           # 4 pairs of 2 batches
ROWS = BPC * C             # 512 rows of (C,T) per core
TP = 4096                  # T padded to a multiple of 128
NKT = TP // 128            # 32 k-tiles
GT = 8                     # transpose tiles per PSUM group
NG = NKT // GT             # 4 groups per pair
GW = GT * 128              # 1024 columns per group
NCHUNK = 8
CHW = T // NCHUNK          # 500 (fits one PSUM bank in f32)
CSPLIT = 2048              # load/cast split point
EPS = 1e-8


def _body(ctx, tc, out_ap, x_ap, idf_ap, idb_ap, alpha, doa, gamma):
    nc = tc.nc

    singles = ctx.enter_context(tc.tile_pool(name="singles", bufs=1))
    xfp = ctx.enter_context(tc.tile_pool(name="xfp", bufs=2))
    xbp = ctx.enter_context(tc.tile_pool(name="xbp", bufs=3))
    xtp = ctx.enter_context(tc.tile_pool(name="xtp", bufs=2))
    obp = ctx.enter_context(tc.tile_pool(name="obp", bufs=2))
    attp = ctx.enter_context(tc.tile_pool(name="attp", bufs=2))
    smalls = ctx.enter_context(tc.tile_pool(name="smalls", bufs=3))

    ps_t = ctx.enter_context(tc.tile_pool(name="ps_t", bufs=2, space="PSUM"))
    ps_g = ctx.enter_context(tc.tile_pool(name="ps_g", bufs=2, space="PSUM"))
    ps_o = ctx.enter_context(tc.tile_pool(name="ps_o", bufs=3, space="PSUM"))
    ps_s = ctx.enter_context(tc.tile_pool(name="ps_s", bufs=1, space="PSUM"))

    st = [{} for _ in range(PAIRS)]

    def ld(p):
        v = st[p]
        rows = slice(p * 128, (p + 1) * 128)
        x_f32 = xfp.tile([128, T], F32)
        if p == 0:
            # head-critical: quarter loads so cast+transpose start early
            for q in range(4):
                lo, hi = q * GW, min((q + 1) * GW, T)
                nc.sync.dma_start(x_f32[:, lo:hi], x_ap[rows, lo:hi])
        else:
            nc.sync.dma_start(x_f32[:, 0:CSPLIT], x_ap[rows, 0:CSPLIT])
            nc.sync.dma_start(x_f32[:, CSPLIT:T], x_ap[rows, CSPLIT:T])
        v["x_f32"] = x_f32

    # identities and constants ride the ACT ring so the SP ring is pure loads
    ident_f32 = singles.tile([128, 128], F32)
    ident_bf = singles.tile([128, 128], BF16)
    ones_row = singles.tile([1, 128], BF16)

    def preamble():
        nc.scalar.dma_start(ident_bf[:], idb_ap)
        nc.scalar.dma_start(ident_f32[:], idf_ap)
        nc.vector.memset(ones_row[:], 1.0)
        warm_act = singles.tile([1, 2], F32)
        nc.scalar.activation(
            warm_act[:], ones_row[0:1, 0:2], mybir.ActivationFunctionType.Exp
        )

    def cast(p):
        """f32 -> bf16 in GW-aligned quarters (DVE/ACT alternating), each
        accumulating its share of the row sums s.  Transpose group g only
        waits on quarter g."""
        v = st[p]
        x_f32 = v["x_f32"]
        x_bf = xbp.tile([128, TP], BF16)
        s_ab = smalls.tile([128, 4], F32)
        nc.gpsimd.memset(x_bf[:, T:TP], 0.0)
        for q in range(4):
            lo, hi = q * GW, min((q + 1) * GW, T)
            if q % 2 == 0:
                nc.vector.tensor_scalar(
                    x_bf[:, lo:hi], x_f32[:, lo:hi], scalar1=1.0, scalar2=0.0,
                    op0=AluOpType.mult, op1=AluOpType.add,
                    accum_out=s_ab[:, q:q + 1],
                )
            else:
                nc.scalar.activation(
                    x_bf[:, lo:hi], x_f32[:, lo:hi],
                    mybir.ActivationFunctionType.Copy,
                    accum_out=s_ab[:, q:q + 1],
                )
        s_col = smalls.tile([128, 1], F32)
        nc.vector.tensor_reduce(s_col[:], s_ab[:], axis=AX, op=AluOpType.add)
        v.update(x_bf=x_bf, s_col=s_col)

    def tgrp(p, g):
        """8 PE transposes into one PSUM bank, then one batched copy out."""
        v = st[p]
        x_bf = v["x_bf"]
        if g == 0:
            xt = xtp.tile([128, TP], BF16)
            v["xt"] = xt
        ps = ps_t.tile([128, GW], BF16, tag="t")
        for j in range(GT):
            kt = g * GT + j
            nc.tensor.transpose(
                ps[:, j * 128:(j + 1) * 128],
                x_bf[:, kt * 128:(kt + 1) * 128],
                ident_bf[:],
            )
        dst = v["xt"][:, g * GW:(g + 1) * GW]
        if g % 2 == 0:
            nc.vector.tensor_copy(dst, ps[:])
        else:
            nc.scalar.copy(dst, ps[:])

    def s_row(p):
        """s-col PE transpose + scaled aux row (tiny, feeds the aux matmul)."""
        v = st[p]
        st_ps = ps_s.tile([1, 128], F32, tag="st")
        nc.tensor.transpose(st_ps[:], v["s_col"][:], ident_f32[:])
        rhs_aux = smalls.tile([1, 128], BF16)
        nc.vector.tensor_scalar_mul(rhs_aux[:], st_ps[:], doa)
        v["rhs_aux"] = rhs_aux

    def g8(p, g):
        """one group of 8 Gram matmuls; g==0 opens the PSUM accum group."""
        v = st[p]
        xt = v["xt"]
        if g == 0:
            psum_g = ps_g.tile([128, 128], F32, tag="g")
            v["psum_g"] = psum_g
        psum_g = v["psum_g"]
        for j in range(GT):
            base = (g * GT + j) * 128
            nc.tensor.matmul(
                psum_g[:],
                lhsT=xt[:, base: base + 128],
                rhs=xt[:, base: base + 128],
                start=(g == 0 and j == 0),
                stop=False,
                skip_group_check=True,
            )

    def aux(p):
        v = st[p]
        nc.tensor.matmul(
            v["psum_g"][:],
            lhsT=ones_row[:],
            rhs=v["rhs_aux"][:],
            start=False,
            stop=True,
            skip_group_check=True,
        )

    def smax(p):
        """energy extraction + min-max softmax -> attention lhsT (+identity)."""
        v = st[p]
        psum_g = v["psum_g"]
        e_sb = smalls.tile([128, 64], F32)
        nc.vector.tensor_scalar_mul(e_sb[0:64, :], psum_g[0:64, 0:64], alpha)
        nc.vector.tensor_scalar_mul(
            e_sb[64:128, :], psum_g[64:128, 64:128], alpha
        )
        rmax = smalls.tile([128, 1], F32)
        nc.vector.tensor_reduce(rmax[:], e_sb[:], axis=AX, op=AluOpType.max)
        rmin = smalls.tile([128, 1], F32)
        nc.vector.tensor_reduce(rmin[:], e_sb[:], axis=AX, op=AluOpType.min)
        den = smalls.tile([128, 1], F32)
        nc.vector.tensor_scalar(
            den[:], rmax[:], scalar1=rmin[:], scalar2=EPS,
            op0=AluOpType.subtract, op1=AluOpType.add,
        )
        rden = smalls.tile([128, 1], F32)
        nc.vector.reciprocal(rden[:], den[:])
        nbias = smalls.tile([128, 1], F32)
        nc.vector.scalar_tensor_tensor(
            nbias[:], in0=rmin[:], scalar=-1.0, in1=rden[:],
            op0=AluOpType.mult, op1=AluOpType.mult,
        )
        ex = smalls.tile([128, 64], F32)
        nc.scalar.activation(
            ex[:], e_sb[:], mybir.ActivationFunctionType.Exp,
            bias=nbias[:], scale=rden[:],
        )
        ssum = smalls.tile([128, 1], F32)
        nc.vector.tensor_reduce(ssum[:], ex[:], axis=AX, op=AluOpType.add)
        rsum = smalls.tile([128, 1], F32)
        nc.vector.reciprocal(rsum[:], ssum[:])

        latt = attp.tile([128, 128], BF16)
        nc.vector.memset(latt[:], 0.0)
        nc.vector.tensor_scalar(
            latt[0:64, 0:64], ex[0:64, :], scalar1=rsum[0:64], scalar2=gamma,
            op0=AluOpType.mult, op1=AluOpType.mult,
        )
        nc.vector.tensor_scalar(
            latt[64:128, 64:128], ex[64:128, :], scalar1=rsum[64:128],
            scalar2=gamma, op0=AluOpType.mult, op1=AluOpType.mult,
        )
        # residual: out = (gamma*A + I)^T @ X
        nc.vector.tensor_add(latt[:], latt[:], ident_bf[:])
        v["latt"] = latt

    def attn(p):
        """attention+residual matmul chunks -> bf16 out tile -> SP-ring store.
        Tail-critical last pair stores in quarters."""
        v = st[p]
        rows = slice(p * 128, (p + 1) * 128)
        x_bf, latt = v["x_bf"], v["latt"]
        out_bf = obp.tile([128, T], BF16)
        store_after = {1: (0, 1000), 3: (1000, 2000), 5: (2000, 3000),
                       7: (3000, T)} if p == PAIRS - 1 else \
                      {3: (0, 2000), 7: (2000, T)}
        for ch in range(NCHUNK):
            cols = slice(ch * CHW, (ch + 1) * CHW)
            psum_o = ps_o.tile([128, CHW], F32, tag="o")
            nc.tensor.matmul(
                psum_o[:], lhsT=latt[:], rhs=x_bf[:, cols], start=True,
                stop=True,
            )
            if ch % 2 == 0:
                nc.vector.tensor_copy(out_bf[:, cols], psum_o[:])
            else:
                nc.scalar.copy(out_bf[:, cols], psum_o[:])
            if ch in store_after:
                lo, hi = store_after[ch]
                nc.sync.dma_start(out_ap[rows, lo:hi], out_bf[:, lo:hi])
        v.clear()

    # software-pipelined schedule, 3 pairs deep.  PE instruction order is
    # kept dense by interleaving transpose groups of pair p with Gram
    # groups of pair p-1, then running attention of pair p-2:
    #   [T(p,g) | G8(p-1,g)]*4, attn(p-2)
    # The batched PSUM->SBUF copies, casts, softmax and out-copies fill
    # DVE/ACT slots concurrently.
    def block(tp, gp, ap):
        if gp is not None:
            s_row(gp)
        for g in range(NG):
            if tp is not None:
                tgrp(tp, g)
            if gp is not None:
                g8(gp, g)
        if gp is not None:
            aux(gp)
        if ap is not None:
            attn(ap)

    sched = [
        (ld, 0), (preamble, None), (cast, 0), (ld, 1),
        (block, (0, None, None)), (cast, 1), (ld, 2),
        (block, (1, 0, None)), (smax, 0), (cast, 2), (ld, 3),
        (block, (2, 1, 0)), (smax, 1), (cast, 3),
        (block, (3, 2, 1)), (smax, 2),
        (block, (None, 3, 2)), (smax, 3),
        (block, (None, None, 3)),
    ]
    for fn, p in sched:
        if p is None:
            fn()
        elif isinstance(p, tuple):
            fn(*p)
        else:
            fn(p)


_MODULE_CACHE = {}


def _build_module(alpha, doa, gamma):
    key = (alpha, doa, gamma)
    if key in _MODULE_CACHE:
        return _MODULE_CACHE[key]
    nc = bass.Bass(
        "TRN2", target_bir_lowering=False, debug=False, num_devices=NCORES
    )
    x_ap = nc.dram_tensor("x", (ROWS, T), F32, kind="ExternalInput").ap()
    idf_ap = nc.dram_tensor("idf", (128, 128), F32, kind="ExternalInput").ap()
    idb_ap = nc.dram_tensor("idb", (128, 128), BF16, kind="ExternalInput").ap()
    out_ap = nc.dram_tensor("out", (ROWS, T), BF16, kind="ExternalOutput").ap()
    with tile.TileContext(nc) as tc, ExitStack() as ctx:
        _body(ctx, tc, out_ap, x_ap, idf_ap, idb_ap, alpha, doa, gamma)
    if _LEGALIZE_WAITS:
        _split_waits(nc)
    _MODULE_CACHE[key] = nc
    return nc


# The wait-split legalization confuses CoreSim's bookkeeping (hand-built
# NoOps bypass nc.inst_map); tests flip this off for simulation runs.
_LEGALIZE_WAITS = True


def _split_waits(nc):
    """walrus TRN2 codegen allows only ONE sync wait per instruction; when
    Tile emits more (e.g. PSUM slot reuse: previous-writer completion +
    previous-reader), hoist the extras onto same-engine NoOps inserted
    immediately before — the sequencer dispatches in order, so the blocking
    semantics are identical."""
    nid = [0]
    for f in nc.m.functions:
        for block in f.blocks:
            out = []
            for inst in block.instructions:
                si = getattr(inst, "sync_info", None)
                if (
                    si is not None
                    and si.on_wait
                    and len(si.on_wait) > 1
                    and type(inst).__name__ != "InstNoOp"
                ):
                    waits = list(si.on_wait)
                    for w in waits[:-1]:
                        nid[0] += 1
                        out.append(
                            mybir.InstNoOp(
                                name=f"{inst.name}-wsplit{nid[0]}",
                                engine=inst.engine,
                                ins=[],
                                outs=[],
                                sync_info=mybir.SyncInfo(
                                    on_wait=[w], on_update=[]
                                ),
                                text_hint="wait-split",
                                bass_nofuse=True,
                            )
                        )
                    inst.sync_info = mybir.SyncInfo(
                        on_wait=waits[-1:], on_update=list(si.on_update)
                    )
                out.append(inst)
            block.instructions[:] = out


def _prepare(inputs):
    x = np.ascontiguousarray(
        np.asarray(inputs["x"], dtype=np.float32).reshape(B * C, T)
    )
    w1 = np.asarray(inputs["w1"], dtype=np.float64)
    b1 = np.asarray(inputs["b1"], dtype=np.float64)
    w2 = np.asarray(inputs["w2"], dtype=np.float64)
    b2 = np.asarray(inputs["b2"], dtype=np.float64)
    gamma = float(np.asarray(inputs["gamma"]))
    alpha = float(w1 @ w2)
    delta = float(b1 @ w2)
    assert abs(alpha) > 1e-12, "degenerate alpha not supported"
    nc = _build_module(alpha, delta / alpha, gamma)
    ident_f = np.eye(128, dtype=np.float32)
    ident_b = np.eye(128, dtype=ml_dtypes.bfloat16)
    in_maps = [
        {"x": x[i * ROWS:(i + 1) * ROWS], "idf": ident_f, "idb": ident_b}
        for i in range(NCORES)
    ]
    return nc, in_maps


def kernel(**inputs):
    nc, in_maps = _prepare(inputs)
    res = run_bass_kernel_spmd(nc, in_maps, core_ids=list(range(NCORES)))
    out = np.concatenate(
        [np.asarray(res.results[i]["out"]) for i in range(NCORES)], axis=0
    ).astype(np.float32)
    return out.reshape(B, C, T, 1)
